# revision 29
# baseline (speedup 1.0000x reference)
"""ConvAttention Trainium2 kernel (Bass/Tile), data-parallel over batch on 8
NeuronCores (1 batch per core, weights broadcast).

Reference computation (per batch b):
  q = conv3d(input, wq, 1x3x3, pad (0,1,1)) + bq, scaled by 0.5
  k = conv3d(memory, wk, 1x3x3, pad (0,1,1)) + bk
  v = conv3d(memory, wv, 3x3x3, pad (0,1,1)) + bv        (depth valid: L-2)
  heads split depth: q,k -> (2, 128, 9*32*32), v -> (2, 128, 8*32*32)
  logit[h] = q[h] @ k[h].T -> softmax over last axis -> @ v[h]
  out (128, 16, 32, 32)

Kernel design per core:
  - Convs as shifted matmuls over zero-padded [Cin, 34, 34] depth-slice
    images streamed slice by slice. Partitions 64..127 hold a copy shifted
    one column left (built on-chip via SBUF->SBUF DMA), so two 3x3 taps pack
    into one K=128 matmul; leftover dx=2 taps run as K=64 matmuls packed
    pairwise onto disjoint PE row groups (concurrent, different PSUM banks).
    Total conv matmuls = 756 = the K=128 MAC-bound minimum.
  - q/k convs run in fp32r (full PE rate; operands must be DMA-produced),
    v conv shares the same fp32r memory tiles. fp32 accumulation in PSUM.
  - q,k conv outputs evicted (bias fused) to fp16 and transposed to
    spatial-major with ONE blocked DMA-XBAR transpose per [128, 1024] slice
    (out[p, j, c] = in[c, j*128+p]); zero PE/DVE cost. The XBAR queue
    (nc.scalar) must carry ONLY transposes - mixing copy-mode DMAs on that
    queue corrupts transfers.
  - logits accumulate over each head's 9 depth slices into a persistent
    PSUM bank (72 fp16 K=128 matmuls per head). Each slice's logit matmuls
    are DEFERRED one iteration so the XBAR transposes hide behind the next
    slice's conv work (removes ~2us/slice of PE idle; HW then matches the
    cost model at ~314us/iter). Softmax in fp32 via DVE reduce + ACT exp;
    attn cast to fp16, transposed via the XBAR.
  - attn @ V in fp16 (N=512 chunks); output stored fp16, upcast on host.
  - Head 0's attention epilogue is emitted mid-loop (l==9) so it overlaps
    head 1's conv work.

Timing note: per-iteration HW time is measured in test.py with a hardware
For_i loop (reps=257 vs 1) to cancel the axon dispatch overhead.
"""
import numpy as np

import concourse.bacc as bacc
import concourse.mybir as mybir
import concourse.tile as tile
from concourse import bass_utils

F32 = mybir.dt.float32
F32R = mybir.dt.float32r
F16 = mybir.dt.float16

B, CIN, COUT, L, H, W = 8, 64, 128, 18, 32, 32
NH = 2              # heads
DQ = L // NH        # 9 depth slices per head for q/k
LV = L - 2          # 16 v depth slices
DV = LV // NH       # 8 per head
HP, WP = H + 2, W + 2          # padded spatial
SLICE = HP * WP                # 1156
NPOS = H * W                   # 1024 positions per depth slice
DEPTH_SCALE = 0.5

_CACHE = {}


def build_module(reps=1, dma_transpose=True, on_chip_dup=True,
                 do_trans=True, do_logit=True, do_attn=True,
                 qk_f32r=True, split_logits=True,
                 share_w=False, v_f16=False, early_attn=True):
    """reps>1 wraps the whole computation in a hardware loop — used only for
    timing (amortizes the per-dispatch overhead of the execution path)."""
    nc = bacc.Bacc("TRN2", target_bir_lowering=False, debug=False)

    CDT = F32R if qk_f32r else F16   # conv input/weight dtype
    in_parts = 64 if on_chip_dup else 128
    xa = nc.dram_tensor("xa", [in_parts, L, SLICE], CDT, kind="ExternalInput").ap()
    ma = nc.dram_tensor("ma", [in_parts, L, SLICE], CDT, kind="ExternalInput").ap()
    # pair weights: [K=128(2 taps x 64ch), pass, M=128]
    wqp = nc.dram_tensor("wqp", [128, 3, 128], CDT, kind="ExternalInput").ap()
    wkp = nc.dram_tensor("wkp", [128, 3, 128], CDT, kind="ExternalInput").ap()
    # packed dx=2 singles: rows 0..63 = wq tap (dy,2), rows 64..127 = wk tap (dy,2)
    wqks = nc.dram_tensor("wqks", [128, 3, 128], CDT, kind="ExternalInput").ap()
    VDT = F16 if v_f16 else CDT
    wvp = nc.dram_tensor("wvp", [128, 9, 128], VDT, kind="ExternalInput").ap()
    # v dx=2 singles duplicated in both partition halves
    wvs2 = nc.dram_tensor("wvs2", [128, 9, 128], VDT, kind="ExternalInput").ap()
    bq = nc.dram_tensor("bq", [128, 1], F32, kind="ExternalInput").ap()
    bk = nc.dram_tensor("bk", [128, 1], F32, kind="ExternalInput").ap()
    bv = nc.dram_tensor("bv", [128, 1], F32, kind="ExternalInput").ap()
    out = nc.dram_tensor("out", [128, LV * NPOS], F16, kind="ExternalOutput").ap()

    with tile.TileContext(nc) as tc:
        with tc.tile_pool(name="consts", bufs=1) as cpool, \
             tc.tile_pool(name="xin", bufs=6) as xin_pool, \
             tc.tile_pool(name="xmem", bufs=8) as xmem_pool, \
             tc.tile_pool(name="qkc", bufs=8) as qkc_pool, \
             tc.tile_pool(name="qkT", bufs=8) as qkT_pool, \
             tc.tile_pool(name="vall", bufs=1) as vall_pool, \
             tc.tile_pool(name="sm", bufs=2) as sm_pool, \
             tc.tile_pool(name="ost", bufs=3) as ost_pool, \
             tc.tile_pool(name="pconv", bufs=6 if dma_transpose else 4,
                          space="PSUM") as pconv, \
             tc.tile_pool(name="ptrans", bufs=2, space="PSUM") as ptrans, \
             tc.tile_pool(name="plogit", bufs=1, space="PSUM") as plogit:

            wqp_t = cpool.tile([128, 3, 128], CDT)
            wkp_t = cpool.tile([128, 3, 128], CDT)
            wqks_t = cpool.tile([128, 3, 128], CDT)
            wvp_t = cpool.tile([128, 9, 128], VDT)
            wvs2_t = cpool.tile([128, 9, 128], VDT)
            if not dma_transpose:
                id_t = cpool.tile([128, 128], F16)
                nc.gpsimd.memset(id_t[:], 0.0)
                from concourse.masks import make_identity
                make_identity(nc, id_t[:], nomemset=True)
            bq_t = cpool.tile([128, 1], F32)
            bk_t = cpool.tile([128, 1], F32)
            bv_t = cpool.tile([128, 1], F32)
            for t, d in [(wqp_t, wqp), (wkp_t, wkp), (wqks_t, wqks),
                         (bq_t, bq), (bk_t, bk), (bv_t, bv)]:
                nc.sync.dma_start(t[:], d)
            # v-conv weights are first needed at slice l=2; keep them on the
            # sync queue (the scalar queue is reserved for XBAR transposes —
            # mixing copy and transpose modes there corrupts transfers)
            for t, d in [(wvp_t, wvp), (wvs2_t, wvs2)]:
                nc.sync.dma_start(t[:], d)

            v_heads = [vall_pool.tile([128, DV * NPOS], F16, name=f"vh{h}")
                       for h in range(NH)]

            import contextlib
            rep_ctx = (tc.For_i(0, reps, 1) if reps > 1
                       else contextlib.nullcontext())
            with rep_ctx:
                logit_ps = [plogit.tile([128, 128], F32, tag="logit",
                                        name=f"logit{h}") for h in range(NH)]

                def load_padded_pair(pool, src, l, tag):
                    """[128, 1156] tile: rows 0..63 = padded slice l from HBM,
                    rows 64..127 = same shifted one column left (SBUF copy)."""
                    t = pool.tile([128, SLICE], CDT, tag=tag, name=tag)
                    if on_chip_dup:
                        nc.sync.dma_start(t[0:64, :], src[:, l])
                        nc.sync.dma_start(t[64:128, 0:SLICE - 1],
                                          t[0:64, 1:SLICE])
                    else:
                        nc.sync.dma_start(t[:], src[:, l])
                    return t

                def conv_qk_slice(qps, kps, in_t, mem_t):
                    """share_w order: both 16-row tiles per weight pass."""
                    xv = in_t[:].rearrange("p (h w) -> p h w", h=HP)
                    mv = mem_t[:].rearrange("p (h w) -> p h w", h=HP)
                    xv64 = in_t[0:64].rearrange("p (h w) -> p h w", h=HP)
                    mv64b = mem_t[64:128].rearrange("p (h w) -> p h w", h=HP)
                    for dy in range(3):
                        for t in range(2):
                            y0 = t * 16
                            nc.tensor.matmul(qps[t][:], wqp_t[:, dy],
                                             xv[:, y0 + dy:y0 + dy + 16, 0:32],
                                             start=(dy == 0), stop=False)
                    for dy in range(3):
                        for t in range(2):
                            y0 = t * 16
                            nc.tensor.matmul(kps[t][:], wkp_t[:, dy],
                                             mv[:, y0 + dy:y0 + dy + 16, 0:32],
                                             start=(dy == 0), stop=False)
                    for dy in range(3):
                        for t in range(2):
                            y0 = t * 16
                            nc.tensor.matmul(qps[t][:], wqks_t[0:64, dy],
                                             xv64[:, y0 + dy:y0 + dy + 16, 2:34],
                                             start=False, stop=(dy == 2))
                            nc.tensor.matmul(kps[t][:], wqks_t[64:128, dy],
                                             mv64b[:, y0 + dy:y0 + dy + 16, 1:33],
                                             start=False, stop=(dy == 2))

                def conv_qk_tile(qp, kp, in_t, mem_t, y0):
                    """q and k conv for one 16-row output tile: 2x3 K=128
                    pair-matmuls + 3 dual K=64 singles on disjoint row groups."""
                    xv = in_t[:].rearrange("p (h w) -> p h w", h=HP)
                    mv = mem_t[:].rearrange("p (h w) -> p h w", h=HP)
                    xv64 = in_t[0:64].rearrange("p (h w) -> p h w", h=HP)
                    mv64b = mem_t[64:128].rearrange("p (h w) -> p h w", h=HP)
                    for dy in range(3):
                        nc.tensor.matmul(qp[:], wqp_t[:, dy],
                                         xv[:, y0 + dy:y0 + dy + 16, 0:32],
                                         start=(dy == 0), stop=False)
                    for dy in range(3):
                        nc.tensor.matmul(kp[:], wkp_t[:, dy],
                                         mv[:, y0 + dy:y0 + dy + 16, 0:32],
                                         start=(dy == 0), stop=False)
                    for dy in range(3):
                        # q single: input top half, dx=2
                        nc.tensor.matmul(qp[:], wqks_t[0:64, dy],
                                         xv64[:, y0 + dy:y0 + dy + 16, 2:34],
                                         start=False, stop=(dy == 2))
                        # k single: memory bottom half (pre-shifted), dx=2
                        nc.tensor.matmul(kp[:], wqks_t[64:128, dy],
                                         mv64b[:, y0 + dy:y0 + dy + 16, 1:33],
                                         start=False, stop=(dy == 2))

                def conv_v_slice(vp0, vp1, m_sls):
                    """v conv for one output depth slice (both 16-row tiles):
                    9 K=128 pair-matmuls per tile + 9 dual K=64 singles."""
                    for dl in range(3):
                        mv = m_sls[dl][:].rearrange("p (h w) -> p h w", h=HP)
                        for dy in range(3):
                            i = dl * 3 + dy
                            nc.tensor.matmul(vp0[:], wvp_t[:, i],
                                             mv[:, dy:dy + 16, 0:32],
                                             start=(i == 0), stop=False)
                            nc.tensor.matmul(vp1[:], wvp_t[:, i],
                                             mv[:, 16 + dy:16 + dy + 16, 0:32],
                                             start=(i == 0), stop=False)
                    for dl in range(3):
                        mv64 = m_sls[dl][0:64].rearrange("p (h w) -> p h w", h=HP)
                        mv64b = m_sls[dl][64:128].rearrange("p (h w) -> p h w", h=HP)
                        for dy in range(3):
                            i = dl * 3 + dy
                            nc.tensor.matmul(vp0[:], wvs2_t[0:64, i],
                                             mv64[:, dy:dy + 16, 2:34],
                                             start=False, stop=(i == 8))
                            nc.tensor.matmul(vp1[:], wvs2_t[64:128, i],
                                             mv64b[:, 16 + dy:16 + dy + 16, 1:33],
                                             start=False, stop=(i == 8))

                def attention_head(h):
                    negmax = sm_pool.tile([128, 1], F32, tag="negmax",
                                          name="negmax")
                    nc.vector.tensor_reduce(negmax[:], logit_ps[h][:],
                                            op=mybir.AluOpType.max,
                                            axis=mybir.AxisListType.X,
                                            negate=True)
                    attn_exp = sm_pool.tile([128, 128], F32, tag="attn_exp",
                                            name="attn_exp")
                    rowsum = sm_pool.tile([128, 1], F32, tag="rowsum",
                                          name="rowsum")
                    nc.scalar.activation(attn_exp[:], logit_ps[h][:],
                                         mybir.ActivationFunctionType.Exp,
                                         bias=negmax[:], scale=1.0,
                                         accum_out=rowsum[:])
                    recip = sm_pool.tile([128, 1], F32, tag="recip",
                                         name="recip")
                    nc.vector.reciprocal(recip[:], rowsum[:])
                    attn16 = sm_pool.tile([128, 128], F16, tag="attn16",
                                          name="attn16")
                    nc.vector.tensor_scalar_mul(attn16[:], attn_exp[:],
                                                recip[:])
                    attnT = sm_pool.tile([128, 128], F16, tag="attnT",
                                         name="attnT")
                    nc.scalar.dma_start(attnT[:], attn16[:], transpose=True)

                    for c in range(16):
                        off = h * DV * NPOS + c * 512
                        po = pconv.tile([128, 512], F32, tag="conv", name="po")
                        nc.tensor.matmul(po[:], attnT[:],
                                         v_heads[h][:, c * 512:(c + 1) * 512],
                                         start=True, stop=True)
                        ot = ost_pool.tile([128, 512], F16, tag="ost",
                                           name="ot")
                        nc.vector.tensor_copy(ot[:], po[:])
                        nc.sync.dma_start(out[:, off:off + 512], ot[:])

                mem_window = {}
                pending_logit = None
                for l in range(L):
                    head = l // DQ
                    in_t = load_padded_pair(xin_pool, xa, l, "xin")
                    mem_t = load_padded_pair(xmem_pool, ma, l, "xmem")
                    if v_f16:
                        m16 = xmem_pool.tile([128, SLICE], F16, tag="m16",
                                             name="m16")
                        nc.vector.tensor_copy(m16[:],
                                              mem_t[:].bitcast(F32))
                        mem_window[l] = m16
                    else:
                        mem_window[l] = mem_t

                    # eviction: hi parts always; lo parts when split_logits
                    qc = qkc_pool.tile([128, NPOS], F16, tag="qkc", name="qc")
                    kc = qkc_pool.tile([128, NPOS], F16, tag="qkc", name="kc")
                    if split_logits:
                        qlo = qkc_pool.tile([128, NPOS], F16, tag="qkc",
                                            name="qlo")
                        klo = qkc_pool.tile([128, NPOS], F16, tag="qkc",
                                            name="klo")
                    if share_w:
                        qps = [pconv.tile([128, 512], F32, tag="conv",
                                          name=f"qp{t}") for t in range(2)]
                        kps = [pconv.tile([128, 512], F32, tag="conv",
                                          name=f"kp{t}") for t in range(2)]
                        conv_qk_slice(qps, kps, in_t, mem_t)
                    for t in range(2):
                        sl = slice(t * 512, (t + 1) * 512)
                        if share_w:
                            qp, kp = qps[t], kps[t]
                        else:
                            qp = pconv.tile([128, 512], F32, tag="conv",
                                            name="qp")
                            kp = pconv.tile([128, 512], F32, tag="conv",
                                            name="kp")
                            conv_qk_tile(qp, kp, in_t, mem_t, t * 16)
                        nc.vector.tensor_scalar_add(qc[:, sl], qp[:], bq_t[:])
                        nc.vector.tensor_scalar_add(kc[:, sl], kp[:], bk_t[:])
                        if split_logits:
                            # lo = (psum + bias) - hi, one fused DVE op each
                            nc.vector.scalar_tensor_tensor(
                                qlo[:, sl], qp[:], bq_t[:], qc[:, sl],
                                op0=mybir.AluOpType.add,
                                op1=mybir.AluOpType.subtract)
                            nc.vector.scalar_tensor_tensor(
                                klo[:, sl], kp[:], bk_t[:], kc[:, sl],
                                op0=mybir.AluOpType.add,
                                op1=mybir.AluOpType.subtract)

                    qT = qkT_pool.tile([128, NPOS], F16, tag="qkT", name="qT")
                    kT = qkT_pool.tile([128, NPOS], F16, tag="qkT", name="kT")
                    if split_logits:
                        qloT = qkT_pool.tile([128, NPOS], F16, tag="qkT",
                                             name="qloT")
                        kloT = qkT_pool.tile([128, NPOS], F16, tag="qkT",
                                             name="kloT")
                    if do_trans:
                        if dma_transpose:
                            # one blocked XBAR transpose per tensor:
                            # out[p, j, c] = in[c, j*128+p]
                            pairs = [(qc, qT), (kc, kT)]
                            if split_logits:
                                pairs += [(qlo, qloT), (klo, kloT)]
                            for src_t, dst_t in pairs:
                                nc.scalar.dma_start_transpose(
                                    dst_t[:].rearrange("p (j c) -> p j c", j=8),
                                    src_t[:])
                        else:
                            for j in range(8):
                                for src_t, dst_t in ((qc, qT), (kc, kT)):
                                    tp = ptrans.tile([128, 128], F16, tag="tp",
                                                     name="tp")
                                    nc.tensor.transpose(
                                        tp[:],
                                        src_t[:, j * 128:(j + 1) * 128],
                                        id_t[:])
                                    nc.vector.tensor_copy(
                                        dst_t[:, j * 128:(j + 1) * 128], tp[:])

                    def emit_logits(lslice, a, b, alo=None, blo=None):
                        hd = lslice // DQ
                        first = (lslice % DQ) == 0
                        last = (lslice % DQ) == DQ - 1
                        for j in range(8):
                            js = slice(j * 128, (j + 1) * 128)
                            terms = [(a, b)]
                            if split_logits:
                                terms += [(a, blo), (alo, b)]
                            for ti, (lt, rt) in enumerate(terms):
                                nc.tensor.matmul(
                                    logit_ps[hd][:], lt[:, js], rt[:, js],
                                    start=(first and j == 0 and ti == 0),
                                    stop=(last and j == 7
                                          and ti == len(terms) - 1),
                                    skip_group_check=True)

                    # defer this slice's logit matmuls by one iteration so the
                    # DMA transposes have a full slice of conv work to hide
                    # behind; flush the previous slice's logits now
                    if do_logit:
                        if pending_logit is not None:
                            emit_logits(*pending_logit)
                        if split_logits:
                            pending_logit = (l, qT, kT, qloT, kloT)
                        else:
                            pending_logit = (l, qT, kT)
                        if l == L - 1:
                            emit_logits(*pending_logit)
                            pending_logit = None

                    # v conv for output slice l-2 first: fills the PE while
                    # the qT/kT DMA transposes are in flight
                    if l >= 2:
                        lv = l - 2
                        m_sls = [mem_window[lv], mem_window[lv + 1],
                                 mem_window[lv + 2]]
                        vp0 = pconv.tile([128, 512], F32, tag="conv", name="vp0")
                        vp1 = pconv.tile([128, 512], F32, tag="conv", name="vp1")
                        conv_v_slice(vp0, vp1, m_sls)
                        vh, vd = lv // DV, lv % DV
                        nc.vector.tensor_scalar_add(
                            v_heads[vh][:, vd * NPOS:vd * NPOS + 512],
                            vp0[:], bv_t[:])
                        nc.vector.tensor_scalar_add(
                            v_heads[vh][:, vd * NPOS + 512:(vd + 1) * NPOS],
                            vp1[:], bv_t[:])
                        del mem_window[lv]

                    # head 0's logits finish at l=8 and its v slices (0..7)
                    # at this iteration's v-conv (lv=7, l=9): emit its
                    # attention epilogue here so it overlaps head-1 slices.
                    if do_logit and do_attn and early_attn and l == 9:
                        attention_head(0)
                if do_logit and do_attn:
                    if not early_attn:
                        attention_head(0)
                    attention_head(1)
    nc.compile()
    return nc


def prep_inputs(input, memory, wq, bq, wk, bk, wv, bv, qk_f32r=True,
                v_f16=False):
    """Host-side marshalling: padded images + weight packs (fp32 when the
    convs run in fp32r, else fp16)."""
    cdt = np.float32 if qk_f32r else np.float16
    input = np.asarray(input, dtype=np.float32)
    memory = np.asarray(memory, dtype=np.float32)
    wq = np.asarray(wq, dtype=np.float32) * DEPTH_SCALE
    bq = np.asarray(bq, dtype=np.float32) * DEPTH_SCALE
    wk = np.asarray(wk, dtype=np.float32)
    bk = np.asarray(bk, dtype=np.float32)
    wv = np.asarray(wv, dtype=np.float32)
    bv = np.asarray(bv, dtype=np.float32)

    def padded(x):
        p = np.zeros((B, CIN, L, HP, WP), cdt)
        p[:, :, :, 1:H + 1, 1:W + 1] = x.astype(cdt)
        return p.reshape(B, CIN, L, SLICE)

    xa = padded(input)
    ma = padded(memory)

    def pairs_qk(w):
        # [128, 3, 128]: rows 0..63 = tap (dy, 0), rows 64..127 = tap (dy, 1)
        top = w[:, :, 0, :, 0].transpose(1, 2, 0)
        bot = w[:, :, 0, :, 1].transpose(1, 2, 0)
        return np.ascontiguousarray(
            np.concatenate([top, bot], axis=0)).astype(cdt)

    wqp = pairs_qk(wq)
    wkp = pairs_qk(wk)
    wqks = np.ascontiguousarray(np.concatenate(
        [wq[:, :, 0, :, 2].transpose(1, 2, 0),
         wk[:, :, 0, :, 2].transpose(1, 2, 0)], axis=0)).astype(cdt)

    top = wv[:, :, :, :, 0].transpose(1, 2, 3, 0).reshape(CIN, 9, 128)
    bot = wv[:, :, :, :, 1].transpose(1, 2, 3, 0).reshape(CIN, 9, 128)
    vdt = np.float16 if v_f16 else cdt
    wvp = np.ascontiguousarray(
        np.concatenate([top, bot], axis=0)).astype(vdt)
    vs = wv[:, :, :, :, 2].transpose(1, 2, 3, 0).reshape(CIN, 9, 128)
    wvs2 = np.ascontiguousarray(
        np.concatenate([vs, vs], axis=0)).astype(vdt)

    shared = {
        "wqp": wqp, "wkp": wkp, "wqks": wqks, "wvp": wvp, "wvs2": wvs2,
        "bq": bq.reshape(128, 1), "bk": bk.reshape(128, 1),
        "bv": bv.reshape(128, 1),
    }
    return [{"xa": np.ascontiguousarray(xa[b]),
             "ma": np.ascontiguousarray(ma[b]), **shared} for b in range(B)]


QK_F32R = True
SPLIT_LOGITS = False


def kernel(**inputs):
    if "nc" not in _CACHE:
        _CACHE["nc"] = build_module(qk_f32r=QK_F32R, split_logits=SPLIT_LOGITS)
    nc = _CACHE["nc"]
    in_maps = prep_inputs(**inputs, qk_f32r=QK_F32R)
    res = bass_utils.run_bass_kernel_spmd(nc, in_maps, core_ids=list(range(B)))
    out = np.stack([res.results[b]["out"].reshape(COUT, LV, H, W)
                    for b in range(B)])
    return out.astype(np.float32)
